# revision 1
# baseline (speedup 1.0000x reference)
"""Trainium2 Bass kernel for CLIP-style symmetric contrastive loss.

Problem: image_features [8192, 1024] f32, text_features [8192, 1024] f32.
  loss = 0.5 * (CE(logits, diag) + CE(logits.T, diag)),
  logits = cosine_similarity(img, txt) / 0.07.

Distribution: shard image rows across 8 NeuronCores. Each core m computes the
slab S_m = img_n[m] @ txt_n.T / T  ([1024, 8192]) against the full normalized
text matrix, reduces exp(S - C) along rows (local log-sum-exp) and along
columns (partial column sums), and a single [8194]-float AllReduce combines
the column sums plus the per-core scalar partials. Every core then finishes
the scalar loss locally.

The text matrix ships to the device pre-transposed ([D, N], bf16) so the
contraction dim lands on SBUF partitions with plain contiguous DMAs; its
normalization happens on-device in that layout (ACT squares + PE ones-matmul
partition reduction + per-chunk rsqrt scaling).

Math (C = 1/T upper-bounds every logit, so exp(S - C) <= 1 is stable):
  loss = C + (R + L - (2/T) * Draw) / (2N)
    R    = sum_i log sum_j exp(S_ij - C)
    L    = sum_j log sum_i exp(S_ij - C)
    Draw = sum_i cos(img_i, txt_i)
"""
import threading
from contextlib import ExitStack

import ml_dtypes
import numpy as np

import concourse.bacc as bacc
import concourse.bass as bass
import concourse.bass_isa as bass_isa
import concourse.mybir as mybir
import concourse.tile as tile
from concourse.bass_utils import run_bass_kernel_spmd

F32 = mybir.dt.float32
BF16 = mybir.dt.bfloat16
AF = mybir.ActivationFunctionType
ALU = mybir.AluOpType

N_CORES = 8
N = 8192
D = 1024
TEMPERATURE = 0.07


def build_nc(n=N, d=D, n_cores=N_CORES, no_collective=False, prep_only=False):
    """Build the SPMD Bass program (same program on every core)."""
    inv_t = float(1.0 / TEMPERATURE)
    cexp = float(1.0 / TEMPERATURE)          # stabilizer: max possible logit
    rows = n // n_cores                      # image rows per core
    P = 128
    rp = rows // P                           # row-tiles per core (8)
    kt = d // P                              # contraction tiles (8)
    CH = 512                                 # matmul free-dim chunk
    n_ch = n // CH                           # column chunks (16)
    cb_sz = min(4, n_ch)                     # chunks per psum block

    nc = bacc.Bacc("TRN2", target_bir_lowering=False, debug=False,
                   num_devices=n_cores)
    img = nc.dram_tensor("img", [rows, d], F32, kind="ExternalInput").ap()
    txt_t = nc.dram_tensor("txt_t", [d, n], BF16, kind="ExternalInput").ap()
    txt_own = nc.dram_tensor("txt_own", [rows, d], F32, kind="ExternalInput").ap()
    ones = nc.dram_tensor("ones", [P, P], F32, kind="ExternalInput").ap()
    ones_b = nc.dram_tensor("ones_b", [P, P], BF16, kind="ExternalInput").ap()
    ident = nc.dram_tensor("ident", [P, P], BF16, kind="ExternalInput").ap()
    out = nc.dram_tensor("out", [1, 1], F32, kind="ExternalOutput").ap()

    with tile.TileContext(nc) as tc:
        _body(tc, img, txt_t, txt_own, ones, ones_b, ident, out,
              n=n, d=d, rows=rows, P=P, rp=rp, kt=kt, CH=CH,
              n_ch=n_ch, cb_sz=cb_sz, inv_t=inv_t, cexp=cexp, n_cores=n_cores,
              no_collective=no_collective, prep_only=prep_only)
    nc.compile()
    return nc


def _body(tc, img, txt_t, txt_own, ones, ones_b, ident, out, *, n, d, rows, P,
          rp, kt, CH, n_ch, cb_sz, inv_t, cexp, n_cores, no_collective,
          prep_only):
    nc = tc.nc
    with ExitStack() as ctx:
        persist = ctx.enter_context(tc.tile_pool(name="persist", bufs=1))
        stage_f = ctx.enter_context(tc.tile_pool(name="stage_f", bufs=2))
        stage_b = ctx.enter_context(tc.tile_pool(name="stage_b", bufs=2))
        sqp = ctx.enter_context(tc.tile_pool(name="sqp", bufs=2))
        rbp = ctx.enter_context(tc.tile_pool(name="rbp", bufs=1))
        exp_p = ctx.enter_context(tc.tile_pool(name="exp_p", bufs=4))
        v1 = ctx.enter_context(tc.tile_pool(name="v1", bufs=6))
        csb_p = ctx.enter_context(tc.tile_pool(name="csb_p", bufs=1))
        rpp = ctx.enter_context(tc.tile_pool(name="rpp", bufs=2))
        psum = ctx.enter_context(tc.tile_pool(name="psum", bufs=4, space="PSUM"))
        ssq_ps = ctx.enter_context(tc.tile_pool(name="ssq_ps", bufs=2, space="PSUM"))
        tp_ps = ctx.enter_context(tc.tile_pool(name="tp_ps", bufs=2, space="PSUM"))
        dram = ctx.enter_context(tc.tile_pool(name="dram", bufs=1, space="DRAM"))

        txtT = persist.tile([P, kt, n], BF16, tag="txtT")       # [d-part, k, j]
        imgT = persist.tile([P, kt, rows], BF16, tag="imgT")    # [d-part, k, i]
        acc = persist.tile([P, n], F32, tag="acc")              # col partial sums
        vecs = persist.tile([P, 64], F32, tag="vecs")
        ones_sb = persist.tile([P, P], F32, tag="ones")
        ones_bsb = persist.tile([P, P], BF16, tag="ones_bsb")
        ident_sb = persist.tile([P, P], BF16, tag="ident")
        cs_sb = persist.tile([P, n // P], F32, tag="cs_sb")
        ln_cs = persist.tile([P, n // P], F32, tag="ln_cs")
        ebias = persist.tile([P, 1], F32, tag="ebias")

        cbuf = dram.tile([1, n + 64], F32, tag="cbuf")
        cbuf_out = dram.tile([1, n + 64], F32, tag="cbuf_out", addr_space="Shared")

        nc.sync.dma_start(ones_sb[:], ones[:])
        nc.sync.dma_start(ones_bsb[:], ones_b[:])
        nc.sync.dma_start(ident_sb[:], ident[:])
        nc.gpsimd.memset(ebias[:], float(-cexp))

        # vecs column map:
        RS = 0          # cols 0..rp-1   : per-row-tile rowsum(exp)
        DG = 8          # cols 8..8+rp-1 : per-row-tile diag cosine partials
        LNR = 16        # cols 16..: ln of rowsums
        SC = 56         # col 56: R partial, 57: Draw partial

        # --- Phase A: image prep (+ diag dot with own text rows) ------------
        for t in range(rp):
            img_raw = stage_f.tile([P, d], F32, tag="stage")
            nc.sync.dma_start(img_raw[:], img[t * P:(t + 1) * P, :])
            to_raw = stage_f.tile([P, d], F32, tag="stage")
            nc.sync.dma_start(to_raw[:], txt_own[t * P:(t + 1) * P, :])

            v = v1.tile([P, 8], F32, tag="v1")
            sq = stage_b.tile([P, d], BF16, tag="sq")
            nc.scalar.activation(sq[:], img_raw[:], AF.Square,
                                 accum_out=v[:, 0:1])
            nc.scalar.activation(v[:, 1:2], v[:, 0:1], AF.Sqrt)
            nc.vector.reciprocal(v[:, 2:3], v[:, 1:2])       # 1/||img_i||
            sq2 = stage_b.tile([P, d], BF16, tag="sq")
            nc.scalar.activation(sq2[:], to_raw[:], AF.Square,
                                 accum_out=v[:, 3:4])
            nc.scalar.activation(v[:, 4:5], v[:, 3:4], AF.Sqrt)
            nc.vector.reciprocal(v[:, 5:6], v[:, 4:5])       # 1/||txt_own_i||

            dot_scr = stage_b.tile([P, d], BF16, tag="sq")
            nc.vector.tensor_tensor(dot_scr[:], img_raw[:], to_raw[:], ALU.mult)
            nc.vector.tensor_reduce(v[:, 6:7], dot_scr[:],
                                    axis=mybir.AxisListType.X, op=ALU.add)
            nc.vector.tensor_tensor(v[:, 7:8], v[:, 2:3], v[:, 5:6], ALU.mult)
            nc.vector.tensor_tensor(vecs[:, DG + t:DG + t + 1], v[:, 6:7],
                                    v[:, 7:8], ALU.mult)     # diag cosine

            imgn_b = stage_b.tile([P, d], BF16, tag="nrm")
            nc.vector.tensor_scalar_mul(imgn_b[:], img_raw[:], v[:, 2:3])
            # transpose imgn_b [128 i, 1024 d] into imgT k-tiles via PE
            for k in range(kt):
                tp = tp_ps.tile([P, P], BF16, tag="tp")
                nc.tensor.transpose(tp[:], imgn_b[:, k * P:(k + 1) * P],
                                    ident_sb[:])
                nc.vector.tensor_copy(imgT[:, k, t * P:(t + 1) * P], tp[:])

        # --- Phase B: text load (pre-transposed bf16) + normalize in place --
        for k in range(kt):
            nc.sync.dma_start(txtT[:, k, :], txt_t[k * P:(k + 1) * P, :])
        for c in range(n_ch):
            sl = slice(c * CH, (c + 1) * CH)
            ssq = ssq_ps.tile([P, CH], F32, tag="ssq")
            for k in range(kt):
                sqc = sqp.tile([P, CH], BF16, tag="sqc")
                nc.scalar.activation(sqc[:], txtT[:, k, sl], AF.Square)
                nc.tensor.matmul(ssq[:], ones_bsb[:], sqc[:],
                                 start=(k == 0), stop=(k == kt - 1))
            nrm = rbp.tile([P, CH], BF16, tag="nrm_c")
            nc.scalar.activation(nrm[:], ssq[:], AF.Sqrt)
            rcp = rbp.tile([P, CH], F32, tag="rcp")
            nc.vector.reciprocal(rcp[:], nrm[:])
            rb = rbp.tile([P, CH], BF16, tag="rb")
            nc.vector.tensor_copy(rb[:], rcp[:])
            for k in range(kt):
                nc.vector.tensor_tensor(txtT[:, k, sl], txtT[:, k, sl],
                                        rb[:], ALU.mult)

        if prep_only:
            nc.vector.tensor_reduce(vecs[:, 30:31], txtT[:, 0, 0:CH],
                                    axis=mybir.AxisListType.X, op=ALU.add)
            nc.vector.tensor_reduce(vecs[:, 31:32], imgT[:, 0, 0:CH],
                                    axis=mybir.AxisListType.X, op=ALU.add)
            nc.sync.dma_start(out[0:1, 0:1], vecs[0:1, 30:31])
            return

        # --- Phase C: main matmul + exp + row/col reductions ----------------
        for p in range(rp):
            rparts = rpp.tile([P, n_ch], F32, tag="rp")
            for cb in range(n_ch // cb_sz):
                mms = []
                for _ci in range(cb_sz):
                    mm_t = psum.tile([P, CH], F32, tag="mm")
                    mms.append(mm_t)
                for k in range(kt):
                    for ci in range(cb_sz):
                        c = cb * cb_sz + ci
                        nc.tensor.matmul(
                            mms[ci][:],
                            imgT[:, k, p * P:(p + 1) * P],
                            txtT[:, k, c * CH:(c + 1) * CH],
                            start=(k == 0), stop=(k == kt - 1))
                for ci in range(cb_sz):
                    c = cb * cb_sz + ci
                    ex = exp_p.tile([P, CH], BF16, tag="exp")
                    nc.scalar.activation(ex[:], mms[ci][:], AF.Exp,
                                         bias=ebias[:, 0:1], scale=inv_t,
                                         accum_out=rparts[:, c:c + 1])
                    sl = slice(c * CH, (c + 1) * CH)
                    if p == 0:
                        nc.vector.tensor_copy(acc[:, sl], ex[:])
                    else:
                        nc.vector.tensor_tensor(acc[:, sl], acc[:, sl], ex[:],
                                                ALU.add)
            nc.vector.tensor_reduce(vecs[:, RS + p:RS + p + 1], rparts[:],
                                    axis=mybir.AxisListType.X, op=ALU.add)

        # --- Phase D: local scalars -----------------------------------------
        nc.scalar.activation(vecs[:, LNR:LNR + rp], vecs[:, RS:RS + rp], AF.Ln)
        nc.vector.tensor_reduce(vecs[:, 24:25], vecs[:, LNR:LNR + rp],
                                axis=mybir.AxisListType.X, op=ALU.add)
        nc.gpsimd.partition_all_reduce(vecs[:, SC:SC + 1], vecs[:, 24:25],
                                       channels=P, reduce_op=bass_isa.ReduceOp.add)
        nc.vector.tensor_reduce(vecs[:, 25:26], vecs[:, DG:DG + rp],
                                axis=mybir.AxisListType.X, op=ALU.add)
        nc.gpsimd.partition_all_reduce(vecs[:, SC + 1:SC + 2], vecs[:, 25:26],
                                       channels=P, reduce_op=bass_isa.ReduceOp.add)

        # column partial sums (reduce acc over partitions via ones-matmul)
        for c in range(n_ch):
            ps = psum.tile([P, CH], F32, tag="mm")
            nc.tensor.matmul(ps[:], ones_sb[:], acc[:, c * CH:(c + 1) * CH],
                             start=True, stop=True)
            csb = csb_p.tile([P, CH], F32, tag="csb")
            nc.vector.tensor_copy(csb[0:1, :], ps[0:1, :])
            nc.sync.dma_start(cbuf[0:1, c * CH:(c + 1) * CH], csb[0:1, :])
        nc.sync.dma_start(cbuf[0:1, n:n + 2], vecs[0:1, SC:SC + 2])

        # --- Phase E: AllReduce + finish -------------------------------------
        if no_collective:
            nc.sync.dma_start(cbuf_out[:], cbuf[:])
        else:
            nc.gpsimd.collective_compute(
                "AllReduce", ALU.add,
                replica_groups=[list(range(n_cores))],
                ins=[cbuf[:].opt()], outs=[cbuf_out[:].opt()])

        nc.sync.dma_start(
            cs_sb[:], cbuf_out[0:1, 0:n].rearrange("a (p x) -> (a p) x", p=P))
        nc.scalar.activation(ln_cs[:], cs_sb[:], AF.Ln)
        nc.vector.tensor_reduce(vecs[:, 26:27], ln_cs[:],
                                axis=mybir.AxisListType.X, op=ALU.add)
        nc.gpsimd.partition_all_reduce(vecs[:, 27:28], vecs[:, 26:27],
                                       channels=P, reduce_op=bass_isa.ReduceOp.add)
        rd = v1.tile([P, 8], F32, tag="v1")
        nc.sync.dma_start(rd[0:1, 0:2], cbuf_out[0:1, n:n + 2])

        # loss = cexp + (R + L - (2/T) * Draw) / (2N)
        fin = v1.tile([P, 8], F32, tag="v1")
        nc.vector.tensor_tensor(fin[0:1, 0:1], rd[0:1, 0:1],
                                vecs[0:1, 27:28], ALU.add)          # R + L
        nc.vector.tensor_scalar_mul(fin[0:1, 1:2], rd[0:1, 1:2],
                                    float(-2.0 * inv_t))            # -(2/T) Draw
        nc.vector.tensor_tensor(fin[0:1, 2:3], fin[0:1, 0:1],
                                fin[0:1, 1:2], ALU.add)
        nc.scalar.activation(fin[0:1, 3:4], fin[0:1, 2:3], AF.Copy,
                             bias=float(cexp), scale=float(1.0 / (2 * n)))
        nc.sync.dma_start(out[0:1, 0:1], fin[0:1, 3:4])


def make_in_maps(image_features, text_features, n=N, d=D, n_cores=N_CORES):
    image_features = np.asarray(image_features, dtype=np.float32)
    text_features = np.asarray(text_features, dtype=np.float32)
    rows = n // n_cores
    txt_t = np.ascontiguousarray(text_features.T).astype(ml_dtypes.bfloat16)
    ones = np.ones((128, 128), dtype=np.float32)
    ones_b = np.ones((128, 128), dtype=ml_dtypes.bfloat16)
    ident = np.eye(128, dtype=np.float32).astype(ml_dtypes.bfloat16)
    return [
        {
            "img": image_features[m * rows:(m + 1) * rows],
            "txt_t": txt_t,
            "txt_own": text_features[m * rows:(m + 1) * rows],
            "ones": ones,
            "ones_b": ones_b,
            "ident": ident,
        }
        for m in range(n_cores)
    ]


_CACHE = {}
_LOCK = threading.Lock()


def _get_nc():
    with _LOCK:
        if "nc" not in _CACHE:
            _CACHE["nc"] = build_nc()
        return _CACHE["nc"]


def kernel(image_features, text_features):
    image_features = np.asarray(image_features, dtype=np.float32)
    text_features = np.asarray(text_features, dtype=np.float32)
    assert image_features.shape == (N, D) and text_features.shape == (N, D)
    nc = _get_nc()
    in_maps = make_in_maps(image_features, text_features)
    res = run_bass_kernel_spmd(nc, in_maps, list(range(N_CORES)))
    val = np.float32(res.results[0]["out"][0, 0])
    return np.array(val, dtype=np.float32)



# revision 8
# speedup vs baseline: 1.3406x; 1.3406x over previous
"""Trainium2 Bass kernel for CLIP-style symmetric contrastive loss.

Problem: image_features [8192, 1024] f32, text_features [8192, 1024] f32.
  loss = 0.5 * (CE(logits, diag) + CE(logits.T, diag)),
  logits = cosine_similarity(img, txt) / 0.07.

Distribution: shard image rows across 8 NeuronCores. Each core m computes the
slab S_m = txt_n @ img_n[m].T  ([8192 j, 1024 i]) — text rows on PSUM
partitions, the core's own image rows on the free axis — using fp8(e4m3)
DoubleRow matmuls (K=256 per pass, 0.5 cycles/row). exp(S - C) is reduced
along the free axis (ACT accum -> per-text-row colsum partials) and
elementwise-accumulated across j-blocks (DVE bf16 -> rowsum for the core's
own image rows). The colsum AllReduce is split in two so most of it overlaps
the tail of the main loop.

Normalization: each core computes 1/||.|| for its OWN 1024 text rows (row-
major ACT square+accum, issued before anything else so the [8192] AllReduce
of reciprocal norms overlaps the big operand loads), a gpsimd
partition_broadcast replicates them across partitions, and a DVE+GpSimd pass
rescales the host-shipped raw fp8 text in place. sqrt(1/T) is folded into
both image and text scales so ACT-Exp runs with scale=1.

Math (C = 1/T upper-bounds every logit, so exp(S - C) <= 1 is stable):
  loss = C + (R + L - (2/T) * Draw) / (2N)
    R    = sum_i log sum_j exp(S_ij - C)   (own-i partials, AllReduced)
    L    = sum_j log sum_i exp(S_ij - C)   (colsums AllReduced, ln locally)
    Draw = sum_i cos(img_i, txt_i)         (f32, own rows, AllReduced)
"""
import threading
from contextlib import ExitStack

import ml_dtypes
import numpy as np

import concourse.bacc as bacc
import concourse.bass as bass
import concourse.bass_isa as bass_isa
import concourse.mybir as mybir
import concourse.tile as tile
from concourse.bass_utils import run_bass_kernel_spmd

F32 = mybir.dt.float32
BF16 = mybir.dt.bfloat16
FP8 = mybir.dt.float8e4
AF = mybir.ActivationFunctionType
ALU = mybir.AluOpType
DR = mybir.MatmulPerfMode.DoubleRow

N_CORES = 8
N = 8192
D = 1024
TEMPERATURE = 0.07


def build_nc(n=N, d=D, n_cores=N_CORES, no_collective=False):
    inv_t = float(1.0 / TEMPERATURE)
    rows = n // n_cores                      # image/text rows per core (1024)
    P = 128
    rp = rows // P                           # row tiles per core (8)
    kt = d // P                              # 128-deep k tiles (8)
    nc = bacc.Bacc("TRN2", target_bir_lowering=False, debug=False,
                   num_devices=n_cores)
    img = nc.dram_tensor("img", [rows, d], F32, kind="ExternalInput").ap()
    txt_own = nc.dram_tensor("txt_own", [rows, d], F32, kind="ExternalInput").ap()
    imgT = nc.dram_tensor("imgT", [kt * P, rows], BF16, kind="ExternalInput").ap()
    txt8 = nc.dram_tensor("txt8", [kt * P, n], FP8, kind="ExternalInput").ap()
    ones_b = nc.dram_tensor("ones_b", [P, P], BF16, kind="ExternalInput").ap()
    rmask = nc.dram_tensor("rmask", [P, n // P], F32, kind="ExternalInput").ap()
    out = nc.dram_tensor("out", [1, 1], F32, kind="ExternalOutput").ap()

    with tile.TileContext(nc) as tc:
        _body(tc, img, txt_own, imgT, txt8, ones_b, rmask, out,
              n=n, d=d, rows=rows, P=P, rp=rp, kt=kt, inv_t=inv_t,
              n_cores=n_cores, no_collective=no_collective)
    nc.compile()
    return nc


def _body(tc, img, txt_own, imgT, txt8d, ones_b, rmask, out, *, n, d, rows, P,
          rp, kt, inv_t, n_cores, no_collective):
    nc = tc.nc
    cexp = inv_t                 # stabilizer: max possible logit
    sqs = float(np.sqrt(inv_t))  # sqrt(1/T), folded into both scales
    jB = n // P                  # 64 j-blocks of 128 text rows
    JC = 8                       # text scale chunks
    jc_w = n // JC               # 1024 columns per scale chunk
    CI = rows // 512             # image free chunks (2)
    SPLIT = 48                   # j-blocks covered by the first AllReduce
    nA = SPLIT * P               # 6144

    with ExitStack() as ctx:
        persist = ctx.enter_context(tc.tile_pool(name="persist", bufs=1))
        stage_f = ctx.enter_context(tc.tile_pool(name="stage_f", bufs=3))
        stage_b = ctx.enter_context(tc.tile_pool(name="stage_b", bufs=4))
        v1 = ctx.enter_context(tc.tile_pool(name="v1", bufs=4))
        r1p = ctx.enter_context(tc.tile_pool(name="r1p", bufs=2))
        exp_p = ctx.enter_context(tc.tile_pool(name="exp_p", bufs=8))
        ctp = ctx.enter_context(tc.tile_pool(name="ctp", bufs=4))
        psum = ctx.enter_context(tc.tile_pool(name="psum", bufs=8, space="PSUM"))
        dram = ctx.enter_context(tc.tile_pool(name="dram", bufs=1, space="DRAM"))

        txt8s = persist.tile([P, kt, n], FP8, tag="txt8")      # 64KB/part
        img8 = persist.tile([P, kt, rows], FP8, tag="img8")    # 8KB
        imgTs = persist.tile([P, kt, rows], BF16, tag="imgTs")  # 16KB
        rcpt = persist.tile([P, n], F32, tag="rcpt")           # 32KB
        rcpi = persist.tile([P, rows], F32, tag="rcpi")        # 4KB
        acc = persist.tile([P, CI, 512], BF16, tag="acc")      # 2KB
        colacc = persist.tile([P, jB], F32, tag="colacc")
        vecs = persist.tile([P, 40], F32, tag="vecs")
        rfull = persist.tile([P, jB], F32, tag="rfull")
        maskt = persist.tile([P, jB], F32, tag="maskt")
        ones_sb = persist.tile([P, P], BF16, tag="ones")
        ebias = persist.tile([P, 1], F32, tag="ebias")
        csA = persist.tile([P, SPLIT], F32, tag="csA")
        csB = persist.tile([P, jB - SPLIT], F32, tag="csB")
        lnA = persist.tile([P, SPLIT], F32, tag="lnA")
        lnB = persist.tile([P, jB - SPLIT], F32, tag="lnB")

        rbuf = dram.tile([1, n], F32, tag="rbuf")
        rbuf_out = dram.tile([1, n], F32, tag="rbuf_out", addr_space="Shared")
        ibuf = dram.tile([1, rows], F32, tag="ibuf")
        cbufA = dram.tile([1, nA], F32, tag="cbufA")
        cbufA_out = dram.tile([1, nA], F32, tag="cbufA_out", addr_space="Shared")
        cbufB = dram.tile([1, n - nA + 2], F32, tag="cbufB")
        cbufB_out = dram.tile([1, n - nA + 2], F32, tag="cbufB_out",
                              addr_space="Shared")

        # vecs columns: 0..7 img sumsq | 8..15 txt sumsq | 16..23 diag dot
        # 24..31 irc/trc | 32.. scalars
        ISQ, TSQ, DOT, SC = 0, 8, 16, 32

        nc.sync.dma_start(maskt[:], rmask[:])

        # --- Own-text norms first: the rcp AllReduce gates the main loop ----
        for t in range(rp):
            to_raw = stage_f.tile([P, d], F32, tag="stage")
            nc.sync.dma_start(to_raw[:], txt_own[t * P:(t + 1) * P, :])
            sq = stage_b.tile([P, d], BF16, tag="sq")
            nc.scalar.activation(sq[:], to_raw[:], AF.Square,
                                 accum_out=vecs[:, TSQ + t:TSQ + t + 1])

        irc = vecs[:, 24:24 + rp]
        trc = vecs[:, 24 + rp:24 + 2 * rp]
        nrm = v1.tile([P, 2 * rp], F32, tag="nrm")
        nc.scalar.activation(nrm[:, 0:rp], vecs[:, TSQ:TSQ + rp], AF.Sqrt)
        nc.vector.reciprocal(trc, nrm[:, 0:rp])
        trcs = v1.tile([P, rp], F32, tag="v1s")
        nc.vector.tensor_scalar_mul(trcs[:], trc, sqs)
        for c in range(jB // rp):
            nc.vector.tensor_copy(rfull[:, c * rp:(c + 1) * rp], trcs[:])
        nc.vector.tensor_tensor(rfull[:], rfull[:], maskt[:], ALU.mult)
        nc.sync.dma_start(
            rbuf[0:1, :].rearrange("a (x p) -> (a p) x", p=P), rfull[:])
        if no_collective:
            nc.sync.dma_start(rbuf_out[:], rbuf[:])
        else:
            nc.gpsimd.collective_compute(
                "AllReduce", ALU.add,
                replica_groups=[list(range(n_cores))],
                ins=[rbuf[:].opt()], outs=[rbuf_out[:].opt()])

        # --- Big operand loads + image prep (overlap the collective) --------
        nc.sync.dma_start(ones_sb[:], ones_b[:])
        nc.gpsimd.memset(ebias[:], float(-cexp))
        for k in range(kt):
            nc.sync.dma_start(txt8s[:, k, :], txt8d[k * P:(k + 1) * P, :])
            nc.sync.dma_start(imgTs[:, k, :], imgT[k * P:(k + 1) * P, :])

        for t in range(rp):
            img_raw = stage_f.tile([P, d], F32, tag="stage")
            nc.sync.dma_start(img_raw[:], img[t * P:(t + 1) * P, :])
            to2 = stage_f.tile([P, d], F32, tag="stage")
            nc.sync.dma_start(to2[:], txt_own[t * P:(t + 1) * P, :])
            sq2 = stage_b.tile([P, d], BF16, tag="sq")
            nc.scalar.activation(sq2[:], img_raw[:], AF.Square,
                                 accum_out=vecs[:, ISQ + t:ISQ + t + 1])
            dots = stage_b.tile([P, d], BF16, tag="sq")
            nc.vector.tensor_tensor(dots[:], img_raw[:], to2[:], ALU.mult)
            nc.vector.tensor_reduce(vecs[:, DOT + t:DOT + t + 1], dots[:],
                                    axis=mybir.AxisListType.X, op=ALU.add)

        nc.scalar.activation(nrm[:, rp:2 * rp], vecs[:, ISQ:ISQ + rp], AF.Sqrt)
        nc.vector.reciprocal(irc, nrm[:, rp:2 * rp])
        ircs = v1.tile([P, rp], F32, tag="v1s")
        nc.vector.tensor_scalar_mul(ircs[:], irc, sqs)

        # diag cosine partial
        dg = v1.tile([P, rp + 1], F32, tag="v1s")
        nc.vector.tensor_tensor(dg[:, 0:rp], vecs[:, DOT:DOT + rp], irc,
                                ALU.mult)
        nc.vector.tensor_tensor(dg[:, 0:rp], dg[:, 0:rp], trc, ALU.mult)
        nc.vector.tensor_reduce(dg[:, rp:rp + 1], dg[:, 0:rp],
                                axis=mybir.AxisListType.X, op=ALU.add)
        nc.gpsimd.partition_all_reduce(vecs[:, SC + 1:SC + 2], dg[:, rp:rp + 1],
                                       channels=P, reduce_op=bass_isa.ReduceOp.add)

        # image reciprocal norms -> all partitions (DRAM bounce + broadcast)
        nc.sync.dma_start(
            ibuf[0:1, :].rearrange("a (x p) -> (a p) x", p=P), ircs[:])
        i1 = r1p.tile([1, rows], F32, tag="r1")
        nc.sync.dma_start(i1[:], ibuf[:])
        nc.gpsimd.partition_broadcast(rcpi[:], i1[:])
        for k in range(kt):
            nc.vector.tensor_tensor(img8[:, k, :], imgTs[:, k, :], rcpi[:],
                                    ALU.mult)

        # text reciprocal norms -> all partitions, chunk by chunk
        for jc in range(JC):
            r1 = r1p.tile([1, jc_w], F32, tag="r1")
            nc.sync.dma_start(r1[:], rbuf_out[0:1, jc * jc_w:(jc + 1) * jc_w])
            nc.gpsimd.partition_broadcast(rcpt[:, jc * jc_w:(jc + 1) * jc_w],
                                          r1[:])

        # --- Main loop: scale chunk (DVE + GpSimd), then its j-blocks -------
        for jc in range(JC):
            sl = slice(jc * jc_w, (jc + 1) * jc_w)
            for k in range(kt):
                eng = nc.vector if k < 5 else nc.gpsimd
                eng.tensor_tensor(txt8s[:, k, sl], txt8s[:, k, sl],
                                  rcpt[:, sl], ALU.mult)
            for jb in range(jc * (jB // JC), (jc + 1) * (jB // JC)):
                mms = []
                for _ci in range(CI):
                    mm_t = psum.tile([P, 512], F32, tag="mm")
                    mms.append(mm_t)
                for t in range(kt // 2):
                    for ci in range(CI):
                        nc.tensor.matmul(
                            mms[ci][:],
                            txt8s[:, 2 * t:2 * t + 2, jb * P:(jb + 1) * P],
                            img8[:, 2 * t:2 * t + 2, ci * 512:(ci + 1) * 512],
                            start=(t == 0), stop=(t == kt // 2 - 1),
                            perf_mode=DR)
                ctmp = ctp.tile([P, CI], F32, tag="ct")
                for ci in range(CI):
                    ex = exp_p.tile([P, 512], BF16, tag="exp")
                    nc.scalar.activation(ex[:], mms[ci][:], AF.Exp,
                                         bias=ebias[:, 0:1], scale=1.0,
                                         accum_out=ctmp[:, ci:ci + 1])
                    if jb == 0:
                        nc.vector.tensor_copy(acc[:, ci, :], ex[:])
                    else:
                        nc.vector.tensor_tensor(acc[:, ci, :], acc[:, ci, :],
                                                ex[:], ALU.add)
                nc.vector.tensor_tensor(colacc[:, jb:jb + 1], ctmp[:, 0:1],
                                        ctmp[:, 1:2], ALU.add)
            if jb == SPLIT - 1:
                # first colsum AllReduce overlaps the last two chunks
                nc.sync.dma_start(
                    cbufA[0:1, :].rearrange("a (x p) -> (a p) x", p=P),
                    colacc[:, 0:SPLIT])
                if no_collective:
                    nc.sync.dma_start(cbufA_out[:], cbufA[:])
                else:
                    nc.gpsimd.collective_compute(
                        "AllReduce", ALU.add,
                        replica_groups=[list(range(n_cores))],
                        ins=[cbufA[:].opt()], outs=[cbufA_out[:].opt()])
                nc.sync.dma_start(
                    csA[:],
                    cbufA_out[0:1, :].rearrange("a (p x) -> (a p) x", p=P))
                laA = ctp.tile([P, 1], F32, tag="laA")
                nc.scalar.activation(lnA[:], csA[:], AF.Ln, accum_out=laA[:])

        # --- Tail: rowsums, scalars, second AllReduce, finish ---------------
        rs = v1.tile([1, CI + 2], F32, tag="rs")
        for ci in range(CI):
            mm = psum.tile([P, 512], F32, tag="mm")
            nc.tensor.matmul(mm[:], ones_sb[:], acc[:, ci, :],
                             start=True, stop=True)
            lnr = exp_p.tile([1, 512], F32, tag="lnr")
            nc.scalar.activation(lnr[:], mm[0:1, :], AF.Ln,
                                 accum_out=rs[0:1, ci:ci + 1])
        sc2 = v1.tile([1, 2], F32, tag="sc2")
        nc.vector.tensor_tensor(sc2[0:1, 0:1], rs[0:1, 0:1], rs[0:1, 1:2],
                                ALU.add)                     # R partial
        nc.vector.tensor_copy(sc2[0:1, 1:2], vecs[0:1, SC + 1:SC + 2])

        nB = n - nA
        nc.sync.dma_start(
            cbufB[0:1, 0:nB].rearrange("a (x p) -> (a p) x", p=P),
            colacc[:, SPLIT:jB])
        nc.sync.dma_start(cbufB[0:1, nB:nB + 2], sc2[0:1, :])
        if no_collective:
            nc.sync.dma_start(cbufB_out[:], cbufB[:])
        else:
            nc.gpsimd.collective_compute(
                "AllReduce", ALU.add,
                replica_groups=[list(range(n_cores))],
                ins=[cbufB[:].opt()], outs=[cbufB_out[:].opt()])

        nc.sync.dma_start(
            csB[:], cbufB_out[0:1, 0:nB].rearrange("a (p x) -> (a p) x", p=P))
        laB = ctp.tile([P, 1], F32, tag="laB")
        nc.scalar.activation(lnB[:], csB[:], AF.Ln, accum_out=laB[:])
        laT = ctp.tile([P, 1], F32, tag="laT")
        nc.vector.tensor_tensor(laT[:], laA[:], laB[:], ALU.add)
        nc.gpsimd.partition_all_reduce(vecs[:, SC:SC + 1], laT[:],
                                       channels=P, reduce_op=bass_isa.ReduceOp.add)
        rd = v1.tile([1, 8], F32, tag="rd")
        nc.sync.dma_start(rd[0:1, 0:2], cbufB_out[0:1, nB:nB + 2])

        # loss = cexp + (R + L - (2/T) * Draw) / (2N)
        fin = v1.tile([1, 8], F32, tag="fin")
        nc.vector.tensor_tensor(fin[0:1, 0:1], rd[0:1, 0:1],
                                vecs[0:1, SC:SC + 1], ALU.add)
        nc.vector.tensor_scalar_mul(fin[0:1, 1:2], rd[0:1, 1:2],
                                    float(-2.0 * inv_t))
        nc.vector.tensor_tensor(fin[0:1, 2:3], fin[0:1, 0:1],
                                fin[0:1, 1:2], ALU.add)
        nc.scalar.activation(fin[0:1, 3:4], fin[0:1, 2:3], AF.Copy,
                             bias=float(cexp), scale=float(1.0 / (2 * n)))
        nc.sync.dma_start(out[0:1, 0:1], fin[0:1, 3:4])


def make_in_maps(image_features, text_features, n=N, d=D, n_cores=N_CORES):
    image_features = np.asarray(image_features, dtype=np.float32)
    text_features = np.asarray(text_features, dtype=np.float32)
    rows = n // n_cores
    P = 128
    kt = d // P
    txt8 = np.ascontiguousarray(text_features.T).astype(
        ml_dtypes.float8_e4m3).reshape(kt * P, n)
    ones_b = np.ones((P, P), dtype=ml_dtypes.bfloat16)
    maps = []
    for m in range(n_cores):
        img_sh = image_features[m * rows:(m + 1) * rows]
        imgT = np.ascontiguousarray(img_sh.T).astype(
            ml_dtypes.bfloat16).reshape(kt * P, rows)
        rmask = np.zeros((P, n // P), dtype=np.float32)
        rmask[:, m * (rows // P):(m + 1) * (rows // P)] = 1.0
        maps.append({
            "img": img_sh,
            "txt_own": text_features[m * rows:(m + 1) * rows],
            "imgT": imgT,
            "txt8": txt8,
            "ones_b": ones_b,
            "rmask": rmask,
        })
    return maps


_CACHE = {}
_LOCK = threading.Lock()


def _get_nc():
    with _LOCK:
        if "nc" not in _CACHE:
            _CACHE["nc"] = build_nc()
        return _CACHE["nc"]


def kernel(image_features, text_features):
    image_features = np.asarray(image_features, dtype=np.float32)
    text_features = np.asarray(text_features, dtype=np.float32)
    assert image_features.shape == (N, D) and text_features.shape == (N, D)
    nc = _get_nc()
    in_maps = make_in_maps(image_features, text_features)
    res = run_bass_kernel_spmd(nc, in_maps, list(range(N_CORES)))
    val = np.float32(res.results[0]["out"][0, 0])
    return np.array(val, dtype=np.float32)


# revision 12
# speedup vs baseline: 1.5667x; 1.1686x over previous
"""Trainium2 Bass kernel for CLIP-style symmetric contrastive loss.

Problem: image_features [8192, 1024] f32, text_features [8192, 1024] f32.
  loss = 0.5 * (CE(logits, diag) + CE(logits.T, diag)),
  logits = cosine_similarity(img, txt) / 0.07.

Distribution: shard image rows across 8 NeuronCores. Each core m computes the
slab S_m = txt_n @ img_n[m].T  ([8192 j, 1024 i]) — text rows on PSUM
partitions, the core's own image rows on the free axis — using fp8(e4m3)
DoubleRow matmuls (K=256 per pass, 0.5 cycles/row). exp(S - C) is reduced
along the free axis (ACT accum -> per-text-row colsum partials) and
elementwise-accumulated across j-blocks (DVE bf16 -> rowsum for the core's
own image rows). The colsum AllReduce is split in two so most of it overlaps
the tail of the main loop.

Normalization: each core computes 1/||.|| for its OWN 1024 text rows (row-
major ACT square+accum, issued before anything else so the [8192] AllReduce
of reciprocal norms overlaps the big operand loads), a gpsimd
partition_broadcast replicates them across partitions, and a DVE+GpSimd pass
rescales the host-shipped raw fp8 text in place. sqrt(1/T) is folded into
both image and text scales so ACT-Exp runs with scale=1.

Math (C = 1/T upper-bounds every logit, so exp(S - C) <= 1 is stable):
  loss = C + (R + L - (2/T) * Draw) / (2N)
    R    = sum_i log sum_j exp(S_ij - C)   (own-i partials, AllReduced)
    L    = sum_j log sum_i exp(S_ij - C)   (colsums AllReduced, ln locally)
    Draw = sum_i cos(img_i, txt_i)         (f32, own rows, AllReduced)
"""
import threading
from contextlib import ExitStack

import ml_dtypes
import numpy as np

import concourse.bacc as bacc
import concourse.bass as bass
import concourse.bass_isa as bass_isa
import concourse.mybir as mybir
import concourse.tile as tile
from concourse.bass_utils import run_bass_kernel_spmd

F32 = mybir.dt.float32
BF16 = mybir.dt.bfloat16
FP8 = mybir.dt.float8e4
AF = mybir.ActivationFunctionType
ALU = mybir.AluOpType
DR = mybir.MatmulPerfMode.DoubleRow

N_CORES = 8
N = 8192
D = 1024
TEMPERATURE = 0.07


def build_nc(n=N, d=D, n_cores=N_CORES, no_collective=False):
    inv_t = float(1.0 / TEMPERATURE)
    rows = n // n_cores                      # image/text rows per core (1024)
    P = 128
    rp = rows // P                           # row tiles per core (8)
    kt = d // P                              # 128-deep k tiles (8)
    nc = bacc.Bacc("TRN2", target_bir_lowering=False, debug=False,
                   num_devices=n_cores)
    img = nc.dram_tensor("img", [rows, d], F32, kind="ExternalInput").ap()
    txt_own = nc.dram_tensor("txt_own", [rows, d], F32, kind="ExternalInput").ap()
    imgT = nc.dram_tensor("imgT", [kt * P, rows], BF16, kind="ExternalInput").ap()
    txt8 = nc.dram_tensor("txt8", [kt * P, n], FP8, kind="ExternalInput").ap()
    ones_b = nc.dram_tensor("ones_b", [P, P], BF16, kind="ExternalInput").ap()
    rmask = nc.dram_tensor("rmask", [P, n // P], F32, kind="ExternalInput").ap()
    out = nc.dram_tensor("out", [1, 1], F32, kind="ExternalOutput").ap()

    with tile.TileContext(nc) as tc:
        _body(tc, img, txt_own, imgT, txt8, ones_b, rmask, out,
              n=n, d=d, rows=rows, P=P, rp=rp, kt=kt, inv_t=inv_t,
              n_cores=n_cores, no_collective=no_collective)
    nc.compile()
    return nc


def _body(tc, img, txt_own, imgT, txt8d, ones_b, rmask, out, *, n, d, rows, P,
          rp, kt, inv_t, n_cores, no_collective):
    nc = tc.nc
    cexp = inv_t                 # stabilizer: max possible logit
    sqs = float(np.sqrt(inv_t))  # sqrt(1/T), folded into both scales
    jB = n // P                  # 64 j-blocks of 128 text rows
    JC = 8                       # text scale chunks
    jc_w = n // JC               # 1024 columns per scale chunk
    CI = rows // 512             # image free chunks (2)
    SPLIT = 48                   # j-blocks covered by the first AllReduce
    nA = SPLIT * P               # 6144

    with ExitStack() as ctx:
        persist = ctx.enter_context(tc.tile_pool(name="persist", bufs=1))
        stage_f = ctx.enter_context(tc.tile_pool(name="stage_f", bufs=3))
        stage_b = ctx.enter_context(tc.tile_pool(name="stage_b", bufs=4))
        v1 = ctx.enter_context(tc.tile_pool(name="v1", bufs=4))
        r1p = ctx.enter_context(tc.tile_pool(name="r1p", bufs=2))
        keepP = ctx.enter_context(tc.tile_pool(name="keepP", bufs=8))
        exp_p = ctx.enter_context(tc.tile_pool(name="exp_p", bufs=8))
        ctp = ctx.enter_context(tc.tile_pool(name="ctp", bufs=4))
        psum = ctx.enter_context(tc.tile_pool(name="psum", bufs=8, space="PSUM"))
        dram = ctx.enter_context(tc.tile_pool(name="dram", bufs=1, space="DRAM"))

        txt8s = persist.tile([P, kt, n], FP8, tag="txt8")      # 64KB/part
        img8 = persist.tile([P, kt, rows], FP8, tag="img8")    # 8KB
        imgTs = persist.tile([P, kt, rows], BF16, tag="imgTs")  # 16KB
        rcpt = persist.tile([P, n], F32, tag="rcpt")           # 32KB
        rcpi = persist.tile([P, rows], F32, tag="rcpi")        # 4KB
        acc = persist.tile([P, CI, 512], BF16, tag="acc")      # 2KB
        colacc = persist.tile([P, jB], F32, tag="colacc")
        vecs = persist.tile([P, 40], F32, tag="vecs")
        rfull = persist.tile([P, jB], F32, tag="rfull")
        maskt = persist.tile([P, jB], F32, tag="maskt")
        ones_sb = persist.tile([P, P], BF16, tag="ones")
        ebias = persist.tile([P, 1], F32, tag="ebias")
        csA = persist.tile([P, SPLIT], F32, tag="csA")
        csB = persist.tile([P, jB - SPLIT], F32, tag="csB")
        lnA = persist.tile([P, SPLIT], F32, tag="lnA")
        lnB = persist.tile([P, jB - SPLIT], F32, tag="lnB")

        rbuf = dram.tile([1, n], F32, tag="rbuf")
        rbuf_out = dram.tile([1, n], F32, tag="rbuf_out", addr_space="Shared")
        ibuf = dram.tile([1, rows], F32, tag="ibuf")
        cbufA = dram.tile([1, nA], F32, tag="cbufA")
        cbufA_out = dram.tile([1, nA], F32, tag="cbufA_out", addr_space="Shared")
        cbufB = dram.tile([1, n - nA + 2], F32, tag="cbufB")
        cbufB_out = dram.tile([1, n - nA + 2], F32, tag="cbufB_out",
                              addr_space="Shared")

        # vecs columns: 0..7 img sumsq | 8..15 txt sumsq | 16..23 diag dot
        # 24..31 irc/trc | 32.. scalars
        ISQ, TSQ, DOT, SC = 0, 8, 16, 32

        nc.sync.dma_start(maskt[:], rmask[:])

        # --- Own-text norms first: the rcp AllReduce gates the main loop ----
        keeps = []
        for t in range(rp):
            to_raw = stage_f.tile([P, d], F32, tag="stage")
            nc.sync.dma_start(to_raw[:], txt_own[t * P:(t + 1) * P, :])
            sq = stage_b.tile([P, d], BF16, tag="sq")
            nc.scalar.activation(sq[:], to_raw[:], AF.Square,
                                 accum_out=vecs[:, TSQ + t:TSQ + t + 1])
            keep_t = keepP.tile([P, d], BF16, tag="keep")
            nc.vector.tensor_copy(keep_t[:], to_raw[:])
            keeps.append(keep_t)

        irc = vecs[:, 24:24 + rp]
        trc = vecs[:, 24 + rp:24 + 2 * rp]
        nrm = v1.tile([P, 2 * rp], F32, tag="nrm")
        nc.scalar.activation(nrm[:, 0:rp], vecs[:, TSQ:TSQ + rp], AF.Sqrt)
        nc.vector.reciprocal(trc, nrm[:, 0:rp])
        trcs = v1.tile([P, rp], F32, tag="v1s")
        nc.vector.tensor_scalar_mul(trcs[:], trc, sqs)
        for c in range(jB // rp):
            nc.vector.tensor_copy(rfull[:, c * rp:(c + 1) * rp], trcs[:])
        nc.vector.tensor_tensor(rfull[:], rfull[:], maskt[:], ALU.mult)
        nc.sync.dma_start(
            rbuf[0:1, :].rearrange("a (x p) -> (a p) x", p=P), rfull[:])
        if no_collective:
            nc.sync.dma_start(rbuf_out[:], rbuf[:])
        else:
            nc.gpsimd.collective_compute(
                "AllReduce", ALU.add,
                replica_groups=[list(range(n_cores))],
                ins=[rbuf[:].opt()], outs=[rbuf_out[:].opt()])

        # --- Big operand loads + image prep (overlap the collective) --------
        nc.sync.dma_start(ones_sb[:], ones_b[:])
        nc.gpsimd.memset(ebias[:], float(-cexp))
        for k in range(kt):
            nc.sync.dma_start(txt8s[:, k, :], txt8d[k * P:(k + 1) * P, :])
            nc.sync.dma_start(imgTs[:, k, :], imgT[k * P:(k + 1) * P, :])

        for t in range(rp):
            img_raw = stage_f.tile([P, d], F32, tag="stage")
            nc.sync.dma_start(img_raw[:], img[t * P:(t + 1) * P, :])
            sq2 = stage_b.tile([P, d], BF16, tag="sq")
            nc.scalar.activation(sq2[:], img_raw[:], AF.Square,
                                 accum_out=vecs[:, ISQ + t:ISQ + t + 1])
            dots = stage_b.tile([P, d], BF16, tag="sq")
            nc.vector.tensor_tensor(dots[:], img_raw[:], keeps[t][:], ALU.mult)
            nc.vector.tensor_reduce(vecs[:, DOT + t:DOT + t + 1], dots[:],
                                    axis=mybir.AxisListType.X, op=ALU.add)

        nc.scalar.activation(nrm[:, rp:2 * rp], vecs[:, ISQ:ISQ + rp], AF.Sqrt)
        nc.vector.reciprocal(irc, nrm[:, rp:2 * rp])
        ircs = v1.tile([P, rp], F32, tag="v1s")
        nc.vector.tensor_scalar_mul(ircs[:], irc, sqs)

        # diag cosine partial
        dg = v1.tile([P, rp + 1], F32, tag="v1s")
        nc.vector.tensor_tensor(dg[:, 0:rp], vecs[:, DOT:DOT + rp], irc,
                                ALU.mult)
        nc.vector.tensor_tensor(dg[:, 0:rp], dg[:, 0:rp], trc, ALU.mult)
        nc.vector.tensor_reduce(dg[:, rp:rp + 1], dg[:, 0:rp],
                                axis=mybir.AxisListType.X, op=ALU.add)
        nc.gpsimd.partition_all_reduce(vecs[:, SC + 1:SC + 2], dg[:, rp:rp + 1],
                                       channels=P, reduce_op=bass_isa.ReduceOp.add)

        # image reciprocal norms -> all partitions (DRAM bounce + broadcast)
        nc.sync.dma_start(
            ibuf[0:1, :].rearrange("a (x p) -> (a p) x", p=P), ircs[:])
        i1 = r1p.tile([1, rows], F32, tag="r1")
        nc.sync.dma_start(i1[:], ibuf[:])
        nc.gpsimd.partition_broadcast(rcpi[:], i1[:])
        for k in range(kt):
            nc.vector.tensor_tensor(img8[:, k, :], imgTs[:, k, :], rcpi[:],
                                    ALU.mult)

        # text reciprocal norms -> all partitions, chunk by chunk
        for jc in range(JC):
            r1 = r1p.tile([1, jc_w], F32, tag="r1")
            nc.sync.dma_start(r1[:], rbuf_out[0:1, jc * jc_w:(jc + 1) * jc_w])
            nc.gpsimd.partition_broadcast(rcpt[:, jc * jc_w:(jc + 1) * jc_w],
                                          r1[:])

        # GpSimd pre-scales the last two chunks (it lags, so give it work
        # that is only needed late); DVE handles chunks 0..5 in the loop.
        for jc in range(6, JC):
            sl = slice(jc * jc_w, (jc + 1) * jc_w)
            for k in range(kt):
                nc.gpsimd.tensor_tensor(txt8s[:, k, sl], txt8s[:, k, sl],
                                        rcpt[:, sl], ALU.mult)

        # --- Main loop: scale chunk (DVE), then its j-blocks ----------------
        for jc in range(JC):
            sl = slice(jc * jc_w, (jc + 1) * jc_w)
            if jc < 6:
                for k in range(kt):
                    nc.vector.tensor_tensor(txt8s[:, k, sl], txt8s[:, k, sl],
                                            rcpt[:, sl], ALU.mult)
            for jb in range(jc * (jB // JC), (jc + 1) * (jB // JC)):
                mms = []
                for _ci in range(CI):
                    mm_t = psum.tile([P, 512], F32, tag="mm")
                    mms.append(mm_t)
                for t in range(kt // 2):
                    for ci in range(CI):
                        nc.tensor.matmul(
                            mms[ci][:],
                            txt8s[:, 2 * t:2 * t + 2, jb * P:(jb + 1) * P],
                            img8[:, 2 * t:2 * t + 2, ci * 512:(ci + 1) * 512],
                            start=(t == 0), stop=(t == kt // 2 - 1),
                            perf_mode=DR)
                ctmp = ctp.tile([P, CI], F32, tag="ct")
                for ci in range(CI):
                    ex = exp_p.tile([P, 512], BF16, tag="exp")
                    nc.scalar.activation(ex[:], mms[ci][:], AF.Exp,
                                         bias=ebias[:, 0:1], scale=1.0,
                                         accum_out=ctmp[:, ci:ci + 1])
                    if jb == 0:
                        nc.vector.tensor_copy(acc[:, ci, :], ex[:])
                    else:
                        nc.vector.tensor_tensor(acc[:, ci, :], acc[:, ci, :],
                                                ex[:], ALU.add)
                nc.vector.tensor_tensor(colacc[:, jb:jb + 1], ctmp[:, 0:1],
                                        ctmp[:, 1:2], ALU.add)
            if jb == SPLIT - 1:
                # first colsum AllReduce overlaps the last two chunks
                nc.sync.dma_start(
                    cbufA[0:1, :].rearrange("a (x p) -> (a p) x", p=P),
                    colacc[:, 0:SPLIT])
                if no_collective:
                    nc.sync.dma_start(cbufA_out[:], cbufA[:])
                else:
                    nc.gpsimd.collective_compute(
                        "AllReduce", ALU.add,
                        replica_groups=[list(range(n_cores))],
                        ins=[cbufA[:].opt()], outs=[cbufA_out[:].opt()])
                nc.sync.dma_start(
                    csA[:],
                    cbufA_out[0:1, :].rearrange("a (p x) -> (a p) x", p=P))
                laA = ctp.tile([P, 1], F32, tag="laA")
                nc.scalar.activation(lnA[:], csA[:], AF.Ln, accum_out=laA[:])

        # --- Tail: rowsums, scalars, second AllReduce, finish ---------------
        rs = v1.tile([1, CI + 2], F32, tag="rs")
        for ci in range(CI):
            mm = psum.tile([P, 512], F32, tag="mm")
            nc.tensor.matmul(mm[:], ones_sb[:], acc[:, ci, :],
                             start=True, stop=True)
            lnr = exp_p.tile([1, 512], F32, tag="lnr")
            nc.scalar.activation(lnr[:], mm[0:1, :], AF.Ln,
                                 accum_out=rs[0:1, ci:ci + 1])
        sc2 = v1.tile([1, 2], F32, tag="sc2")
        nc.vector.tensor_tensor(sc2[0:1, 0:1], rs[0:1, 0:1], rs[0:1, 1:2],
                                ALU.add)                     # R partial
        nc.vector.tensor_copy(sc2[0:1, 1:2], vecs[0:1, SC + 1:SC + 2])

        nB = n - nA
        nc.sync.dma_start(
            cbufB[0:1, 0:nB].rearrange("a (x p) -> (a p) x", p=P),
            colacc[:, SPLIT:jB])
        nc.sync.dma_start(cbufB[0:1, nB:nB + 2], sc2[0:1, :])
        if no_collective:
            nc.sync.dma_start(cbufB_out[:], cbufB[:])
        else:
            nc.gpsimd.collective_compute(
                "AllReduce", ALU.add,
                replica_groups=[list(range(n_cores))],
                ins=[cbufB[:].opt()], outs=[cbufB_out[:].opt()])

        nc.sync.dma_start(
            csB[:], cbufB_out[0:1, 0:nB].rearrange("a (p x) -> (a p) x", p=P))
        laB = ctp.tile([P, 1], F32, tag="laB")
        nc.scalar.activation(lnB[:], csB[:], AF.Ln, accum_out=laB[:])
        laT = ctp.tile([P, 1], F32, tag="laT")
        nc.vector.tensor_tensor(laT[:], laA[:], laB[:], ALU.add)
        nc.gpsimd.partition_all_reduce(vecs[:, SC:SC + 1], laT[:],
                                       channels=P, reduce_op=bass_isa.ReduceOp.add)
        rd = v1.tile([1, 8], F32, tag="rd")
        nc.sync.dma_start(rd[0:1, 0:2], cbufB_out[0:1, nB:nB + 2])

        # loss = cexp + (R + L - (2/T) * Draw) / (2N)
        fin = v1.tile([1, 8], F32, tag="fin")
        nc.vector.tensor_tensor(fin[0:1, 0:1], rd[0:1, 0:1],
                                vecs[0:1, SC:SC + 1], ALU.add)
        nc.vector.tensor_scalar_mul(fin[0:1, 1:2], rd[0:1, 1:2],
                                    float(-2.0 * inv_t))
        nc.vector.tensor_tensor(fin[0:1, 2:3], fin[0:1, 0:1],
                                fin[0:1, 1:2], ALU.add)
        nc.scalar.activation(fin[0:1, 3:4], fin[0:1, 2:3], AF.Copy,
                             bias=float(cexp), scale=float(1.0 / (2 * n)))
        nc.sync.dma_start(out[0:1, 0:1], fin[0:1, 3:4])


def make_in_maps(image_features, text_features, n=N, d=D, n_cores=N_CORES):
    image_features = np.asarray(image_features, dtype=np.float32)
    text_features = np.asarray(text_features, dtype=np.float32)
    rows = n // n_cores
    P = 128
    kt = d // P
    txt8 = np.ascontiguousarray(text_features.T).astype(
        ml_dtypes.float8_e4m3).reshape(kt * P, n)
    ones_b = np.ones((P, P), dtype=ml_dtypes.bfloat16)
    maps = []
    for m in range(n_cores):
        img_sh = image_features[m * rows:(m + 1) * rows]
        imgT = np.ascontiguousarray(img_sh.T).astype(
            ml_dtypes.bfloat16).reshape(kt * P, rows)
        rmask = np.zeros((P, n // P), dtype=np.float32)
        rmask[:, m * (rows // P):(m + 1) * (rows // P)] = 1.0
        maps.append({
            "img": img_sh,
            "txt_own": text_features[m * rows:(m + 1) * rows],
            "imgT": imgT,
            "txt8": txt8,
            "ones_b": ones_b,
            "rmask": rmask,
        })
    return maps


_CACHE = {}
_LOCK = threading.Lock()


def _get_nc():
    with _LOCK:
        if "nc" not in _CACHE:
            _CACHE["nc"] = build_nc()
        return _CACHE["nc"]


def kernel(image_features, text_features):
    image_features = np.asarray(image_features, dtype=np.float32)
    text_features = np.asarray(text_features, dtype=np.float32)
    assert image_features.shape == (N, D) and text_features.shape == (N, D)
    nc = _get_nc()
    in_maps = make_in_maps(image_features, text_features)
    res = run_bass_kernel_spmd(nc, in_maps, list(range(N_CORES)))
    val = np.float32(res.results[0]["out"][0, 0])
    return np.array(val, dtype=np.float32)
